# revision 1
# baseline (speedup 1.0000x reference)
"""Trainium2 Bass kernel: 3x3 stride-1 pad-1 Conv2D forward + bias.

Full problem: x (32,32,128,128) f32, kernels (64,288) f32, bias (64,1) f32
-> out (32,64,128,128) f32.

Sharding: data-parallel over batch — 4 images per core on 8 NeuronCores,
weights/bias replicated. No collectives needed (forward only).

Per-core algorithm:
  The conv is lowered to 3 PSUM-accumulated matmuls per output tile (one per
  kernel column dc), with the contraction packed over (channel, kernel-row):
  PE partition p = 4*c + dr holds the image shifted vertically by dr-1 and
  padded horizontally (width 130), in bf16. Partition 3 is an all-ones row so
  the bias folds into the dc=1 matmul as an extra contraction row; the other
  p%4==3 partitions carry zero weights. Each PSUM bank [128, 512] holds two
  4-row output windows (PE column groups 0/64 via tile_position), accumulated
  over dc, then copied to SBUF (DVE/ACT alternating) and DMA'd out as fp32.

  The host pre-pads/casts x to bf16 [*, 32, 128, 130] so every DMA is
  contiguous per partition; the two shifted copies are built with SBUF->SBUF
  DMAs. The p%4==3 partitions (ones row + zero rows) come from a small
  constant input, loaded once per double-buffer.
"""

import numpy as np
import ml_dtypes

import concourse.bass as bass
import concourse.mybir as mybir
import concourse.tile as tile
from concourse import bacc
from concourse.bass_utils import run_bass_kernel_spmd

N_CORES = 8
B, C, H, W = 32, 32, 128, 128
K = 64
B_LOC = B // N_CORES  # images per core
WP = W + 2  # padded row pitch

DT = "bf16"  # "bf16" | "fp32r"

_DT_MAP = {
    "bf16": (mybir.dt.bfloat16, ml_dtypes.bfloat16),
    "fp32r": (mybir.dt.float32r, np.float32),
}


def _build(
    dt_name: str = DT, reps: int = 1, bench_io: bool = False, ablate: str | None = None
):
    mdt, _ = _DT_MAP[dt_name]
    f32 = mybir.dt.float32

    nc = bacc.Bacc("TRN2", target_bir_lowering=False, debug=False)
    xp = nc.dram_tensor("xp", [B_LOC, C, H, WP], mdt, kind="ExternalInput")
    wt = nc.dram_tensor("wt", [128, 3, K], mdt, kind="ExternalInput")
    aux = nc.dram_tensor("aux", [C, H * WP], mdt, kind="ExternalInput")
    if bench_io:
        # timing variant: big output stays in device DRAM; tiny external out
        out = nc.dram_tensor("obuf", [B_LOC, K, H, W], f32)
        tout = nc.dram_tensor("tout", [1, 1], f32, kind="ExternalOutput")
    else:
        out = nc.dram_tensor("out", [B_LOC, K, H, W], f32, kind="ExternalOutput")

    with tile.TileContext(nc) as tc:
        with (
            tc.tile_pool(name="const", bufs=1) as const_pool,
            tc.tile_pool(name="xrep", bufs=1) as xrep_pool,
            tc.tile_pool(name="psum", bufs=1, space="PSUM") as psum_pool,
            tc.tile_pool(name="ostage", bufs=3) as ostage_pool,
        ):
            wsb = const_pool.tile([128, 3, K], mdt, name="wsb")
            nc.sync.dma_start(wsb[:], wt[:])

            # Three persistent replicated-image buffers (manual multi-buffer:
            # image n+1/n+2 loads overlap image n compute).
            Rs = []
            for i in range(3):
                R = xrep_pool.tile([128, H, WP], mdt, name=f"R{i}", tag=f"R{i}")
                # vertical-pad rows for the shifted groups (p%4==0 keeps row 0
                # zero, p%4==2 keeps row H-1 zero; their loads never write them)
                nc.vector.memset(R[:, 0:1, :], 0.0)
                nc.vector.memset(R[:, H - 1 : H, :], 0.0)
                # ones row (p=3) + zero rows (p=7,11,...) — after the memsets
                # (they must stay ones on rows 0 / H-1); never overwritten
                nc.sync.dma_start(R[3::4, :, :], aux[:])
                Rs.append(R)

            for rep in range(reps):
                for n in range(B_LOC):
                    R = Rs[n % 3]
                    # center copy (dr=1) at partitions 1 mod 4 (contiguous rows)
                    nc.sync.dma_start(R[1::4, :, :], xp[n])
                    # dr=0 group (image shifted down a row): p%4==0
                    nc.scalar.dma_start(R[0::4, 1:H, :], R[1::4, 0 : H - 1, :])
                    # dr=2 group (image shifted up a row): p%4==2
                    nc.scalar.dma_start(R[2::4, 0 : H - 1, :], R[1::4, 1:H, :])

                    # DRAM view: [half, win, k, bank, r, w] matching the
                    # ostage layout (partitions (win,k), free (bank, r, w));
                    # per-(half,win) slices are 3-dim balanceable APs
                    o_n = out[n].rearrange(
                        "k (hf bb win r) w -> hf win k bb r w", hf=2, win=2, r=4
                    )
                    ring = [nc.sync, nc.scalar]
                    for half in range(2):
                        if ablate == "no_mm":
                            continue
                        psums = [
                            psum_pool.tile(
                                [128, 512], f32, name=f"ps{b}", tag=f"ps{b}"
                            )
                            for b in range(8)
                        ]
                        for dc in range(3):
                            for b in range(8):
                                for win in range(2):
                                    v0 = 64 * half + 8 * b + 4 * win
                                    nc.tensor.matmul(
                                        psums[b][64 * win : 64 * win + 64, :],
                                        lhsT=wsb[:, dc, :],
                                        rhs=R[:, v0 : v0 + 4, dc : dc + W],
                                        start=(dc == 0),
                                        stop=(dc == 2),
                                        tile_position=(0, 64 * win),
                                    )
                        ost = ostage_pool.tile(
                            [128, 8, 512], f32, name="ost", tag="ost"
                        )
                        for b in range(8):
                            if ablate == "no_cp" and not (half == 1 and b == 7):
                                continue
                            if b % 2 == 0:
                                nc.vector.tensor_copy(ost[:, b, :], psums[b][:])
                            else:
                                nc.scalar.copy(ost[:, b, :], psums[b][:])
                        if ablate == "no_out" and half != 1:
                            continue
                        for win in range(2):
                            ring[(half + win) % 2].dma_start(
                                o_n[half, win], ost[64 * win : 64 * win + 64, :, :]
                            )
                        last_ost = ost

            if bench_io:
                if ablate == "no_mm":
                    nc.sync.dma_start(tout[:], wsb[0:1, 0, 0:1])
                else:
                    nc.sync.dma_start(tout[:], last_ost[0:1, 0, 0:1])

    nc.compile()
    return nc


def _prep_weights(kernels: np.ndarray, bias: np.ndarray, dt_name: str = DT):
    _, npdt = _DT_MAP[dt_name]
    w4 = kernels.reshape(K, C, 3, 3).astype(np.float32)  # [k, c, dr, dc]
    wt = np.zeros((C, 4, 3, K), np.float32)  # [c, slot(p%4), dc, k]
    wt[:, 0:3, :, :] = np.transpose(w4, (1, 2, 3, 0))  # slot=dr
    wt = wt.reshape(128, 3, K)
    wt[3, 1, :] = bias.reshape(K).astype(np.float32)  # bias row, dc=1 only
    return np.ascontiguousarray(wt.astype(npdt))


def _prep_x(x: np.ndarray, dt_name: str = DT):
    _, npdt = _DT_MAP[dt_name]
    xp = np.zeros((B, C, H, WP), npdt)
    xp[:, :, :, 1 : W + 1] = x.astype(npdt)
    return xp


def _prep_aux(dt_name: str = DT):
    _, npdt = _DT_MAP[dt_name]
    aux = np.zeros((C, H * WP), npdt)
    aux[0, :] = npdt(1.0)  # partition 3 of R = bias ones-row
    return aux


_NC_CACHE: dict[tuple, object] = {}


def _run(x, kernels, bias, dt_name: str = DT, reps: int = 1, trace: bool = False):
    key = (dt_name, reps)
    if key not in _NC_CACHE:
        _NC_CACHE[key] = _build(dt_name, reps)
    nc = _NC_CACHE[key]

    xp = _prep_x(np.asarray(x), dt_name)
    wt = _prep_weights(np.asarray(kernels), np.asarray(bias), dt_name)
    aux = _prep_aux(dt_name)
    in_maps = [
        {"xp": xp[c * B_LOC : (c + 1) * B_LOC], "wt": wt, "aux": aux}
        for c in range(N_CORES)
    ]
    kw = {"trace": True} if trace else {}
    res = run_bass_kernel_spmd(nc, in_maps, list(range(N_CORES)), **kw)
    full = np.concatenate([res.results[c]["out"] for c in range(N_CORES)], axis=0)
    return full, res


def kernel(x, kernels, bias):
    full, _ = _run(x, kernels, bias)
    return full

